# revision 1
# baseline (speedup 1.0000x reference)
"""Cosine-similarity loss kernel for Trainium2 (8 NeuronCores, SPMD).

loss = -sum_n dot(s_n, im_n) / (||s_n|| * ||im_n||)   for s, im in R^{65536 x 512}

Strategy (memory-bound, ~360 GB/s HBM per core):
  - Shard the 65536 rows across 8 cores (8192 rows each, 32 MB/core streamed).
  - Two DMA streams: s via HWDGE (nc.sync), im via SWDGE (nc.gpsimd) -> both
    HW queues run concurrently and saturate HBM (~360 GB/s measured).
  - Per 128-row slice (64 slices/core), three fused one-pass reductions:
      dot = sum_d s*im  -> VectorE scalar_tensor_tensor (s*1)*im, accum_out
      ss  = sum_d s*s   -> ScalarE activation(Square, accum_out)
      ii  = sum_d im*im -> split DVE/ACT to balance (DVE ~695ns, ACT ~903ns/op)
  - Tiny tail: loss_p[128,1] = -sum_slices dot/sqrt(ss*ii); DMA out.
  - Host sums the 8x128 partials -> f32 scalar.
"""

import numpy as np
from contextlib import ExitStack

import concourse.bacc as bacc
import concourse.bass as bass
import concourse.mybir as mybir
import concourse.tile as tile
from concourse.bass_utils import run_bass_kernel_spmd

N, D = 65536, 512
N_CORES = 8
ROWS = N // N_CORES          # 8192 rows per core
P = 128                      # SBUF partitions
F32 = mybir.dt.float32


def _build(
    rows=ROWS,
    # slices per DMA tile (1 slice = 128 rows = 256KB/tensor).  Small first
    # tiles start compute early; small last tiles shrink the post-DMA tail.
    seg_schedule=(1, 1, 2) + (4,) * 14 + (2, 1, 1),
    bufs=10,
    ii_on_act=lambda c: c % 16 >= 11,  # which slices' ii goes to ACT (~5/16)
    split_ii=False,          # separate per-engine ii tiles (kills x-engine WAW)
    im_dma="sync",           # engine issuing im loads: gpsimd | sync | scalar
    mapping="pj",            # row->partition: jp = row j*128+p; pj = p*seg+j
                             # (pj gives contiguous per-partition DMA segments)
):
    slices = rows // P
    assert sum(seg_schedule) == slices

    nc = bacc.Bacc(
        "TRN2", target_bir_lowering=False, debug=False, num_devices=N_CORES
    )
    s_d = nc.dram_tensor("s", [rows, D], F32, kind="ExternalInput").ap()
    im_d = nc.dram_tensor("im", [rows, D], F32, kind="ExternalInput").ap()
    out_d = nc.dram_tensor("out", [P, 1], F32, kind="ExternalOutput").ap()

    mult = mybir.AluOpType.mult

    with tile.TileContext(nc) as tc, ExitStack() as ctx:
        spool = ctx.enter_context(tc.tile_pool(name="spool", bufs=bufs))
        ipool = ctx.enter_context(tc.tile_pool(name="ipool", bufs=bufs))
        stats = ctx.enter_context(tc.tile_pool(name="stats", bufs=1))

        dot_all = stats.tile([P, slices], F32)
        ss_all = stats.tile([P, slices], F32)
        ii_all = stats.tile([P, slices], F32)
        if split_ii:
            ii_act = stats.tile([P, slices], F32)
            nc.vector.memset(ii_all[:], 0.0)
            nc.vector.memset(ii_act[:], 0.0)
        dve_scr = stats.tile([P, D], F32)
        act_scr = stats.tile([P, D], F32)

        c = 0
        r0 = 0
        pat = "(j p) d -> p j d" if mapping == "jp" else "(p j) d -> p j d"
        for seg in seg_schedule:
            nrows = seg * P
            s_seg = s_d[r0 : r0 + nrows, :].rearrange(pat, p=P, j=seg)
            im_seg = im_d[r0 : r0 + nrows, :].rearrange(pat, p=P, j=seg)
            r0 += nrows
            st = spool.tile([P, seg, D], F32, name="st", tag="st")
            nc.sync.dma_start(st[:], s_seg)
            it = ipool.tile([P, seg, D], F32, name="it", tag="it")
            getattr(nc, im_dma).dma_start(it[:], im_seg)
            for j in range(seg):
                nc.vector.scalar_tensor_tensor(
                    out=dve_scr[:], in0=st[:, j, :], scalar=1.0, in1=it[:, j, :],
                    op0=mult, op1=mult,
                    accum_out=dot_all[:, c : c + 1],
                )
                nc.scalar.activation(
                    out=act_scr[:], in_=st[:, j, :],
                    func=mybir.ActivationFunctionType.Square,
                    accum_out=ss_all[:, c : c + 1],
                )
                if ii_on_act(c):
                    nc.scalar.activation(
                        out=act_scr[:], in_=it[:, j, :],
                        func=mybir.ActivationFunctionType.Square,
                        accum_out=(ii_act if split_ii else ii_all)[:, c : c + 1],
                    )
                else:
                    nc.vector.scalar_tensor_tensor(
                        out=dve_scr[:], in0=it[:, j, :], scalar=1.0, in1=it[:, j, :],
                        op0=mult, op1=mult,
                        accum_out=ii_all[:, c : c + 1],
                    )
                c += 1

        # tail: loss_p = -sum_c dot_c * (ss_c*ii_c)^-1/2, via exp(-0.5*ln(x))
        # (Ln+Exp share one ACT table set; Sqrt would force a 2.7us tail
        # table switch).
        if split_ii:
            ii_sum = stats.tile([P, slices], F32)
            nc.vector.tensor_add(ii_sum[:], ii_all[:], ii_act[:])
        else:
            ii_sum = ii_all
        prod = stats.tile([P, slices], F32)
        nc.vector.tensor_tensor(out=prod[:], in0=ss_all[:], in1=ii_sum[:], op=mult)
        lnp = stats.tile([P, slices], F32)
        nc.scalar.activation(lnp[:], prod[:], mybir.ActivationFunctionType.Ln)
        rsq = stats.tile([P, slices], F32)
        nc.scalar.activation(
            rsq[:], lnp[:], mybir.ActivationFunctionType.Exp, scale=-0.5
        )
        fin_scr = stats.tile([P, slices], F32)
        loss_p = stats.tile([P, 1], F32)
        nc.vector.scalar_tensor_tensor(
            out=fin_scr[:], in0=dot_all[:], scalar=-1.0, in1=rsq[:],
            op0=mult, op1=mult,
            accum_out=loss_p[:],
        )
        nc.scalar.dma_start(out_d, loss_p[:])

    nc.compile()
    return nc


_compiled = None


def _get_nc():
    global _compiled
    if _compiled is None:
        _compiled = _build()
    return _compiled


def _run(s, im, nc=None, **kw):
    """Shard, run on 8 cores, return BassKernelResults."""
    s = np.ascontiguousarray(np.asarray(s, dtype=np.float32))
    im = np.ascontiguousarray(np.asarray(im, dtype=np.float32))
    assert s.shape == (N, D) and im.shape == (N, D)
    if nc is None:
        nc = _get_nc()
    in_maps = [
        {"s": s[c * ROWS : (c + 1) * ROWS], "im": im[c * ROWS : (c + 1) * ROWS]}
        for c in range(N_CORES)
    ]
    bkr = run_bass_kernel_spmd(nc, in_maps, core_ids=list(range(N_CORES)), **kw)
    return bkr


def kernel(s, im, temp=None, **_):
    bkr = _run(s, im)
    total = np.float64(0.0)
    for r in bkr.results:
        total += r["out"].astype(np.float64).sum()
    return np.float32(total)



# revision 5
# speedup vs baseline: 1.1132x; 1.1132x over previous
"""Cosine-similarity loss kernel for Trainium2 (8 NeuronCores, SPMD).

loss = -sum_n dot(s_n, im_n) / (||s_n|| * ||im_n||)   for s, im in R^{65536 x 512}

v2 strategy (memory-bound; per-core HBM ~358 GB/s):
  - Host casts inputs to fp16 (measured end-to-end rel err ~2.7e-4, well
    under the 2e-2 gate) -> 16.78 MB/core streamed instead of 33.55 MB.
  - Rows sharded 8 ways; 64 slices of 128 rows per core.  All input DMA
    on the sync HWDGE ring (measured 337 GB/s sustained, zero gaps).
  - Per slice, three one-pass reductions over [128, 512]:
      dot = sum_d s*im   -> DVE scalar_tensor_tensor (fp16 2x mode)
      ss  = sum_d s*s    -> mostly ACT Square+accum, some GPSIMD
      ii  = sum_d im*im  -> split DVE / GPSIMD to balance the 3 engines
  - Tail: rsqrt(ss*ii) = ACT Sqrt(DVE reciprocal(ss*ii)); per-partition
    partials reduced across partitions with a PE ones-matmul into PSUM;
    single [1,1] f32 DMA out (avoids 128x4B scattered-write penalty).
  - Host sums the 8 scalars.
"""

import numpy as np
from contextlib import ExitStack

import concourse.bacc as bacc
import concourse.bass as bass
import concourse.mybir as mybir
import concourse.tile as tile
from concourse.bass_utils import run_bass_kernel_spmd

N, D = 65536, 512
N_CORES = 8
ROWS = N // N_CORES          # 8192 rows per core
P = 128                      # SBUF partitions
SLICES = ROWS // P           # 64
F32 = mybir.dt.float32
F16 = mybir.dt.float16


def _mk_assign(pat_act_ss, pat_gp_ss, pat_gp_ii):
    """Build per-slice engine maps for ss and ii from period-32 patterns."""
    ss_eng, ii_eng = [], []
    for c in range(SLICES):
        m = c % 32
        ss_eng.append("gp" if m in pat_gp_ss else ("act" if m in pat_act_ss else "dve"))
        ii_eng.append("gp" if m in pat_gp_ii else "dve")
    return ss_eng, ii_eng


def _build(
    rows=ROWS,
    seg_schedule=(1, 1, 2, 4, 8, 8, 8, 8, 8, 8, 4, 2, 1, 1),
    bufs=8,
    # GPSIMD shares its SBUF port pair with DVE's 2nd read port (exclusive
    # lock per instruction), so GP compute would serialize against DVE's
    # 2-input ops -> keep GP out.  ss: all ACT; ii: all DVE.
    pat_act_ss=frozenset(range(32)),
    pat_gp_ss=frozenset(),
    pat_gp_ii=frozenset(),
    tail="sqrt",             # sqrt | newton
):
    slices = rows // P
    assert sum(seg_schedule) == slices

    nc = bacc.Bacc(
        "TRN2", target_bir_lowering=False, debug=False, num_devices=N_CORES
    )
    s_d = nc.dram_tensor("s", [rows, D], F16, kind="ExternalInput").ap()
    im_d = nc.dram_tensor("im", [rows, D], F16, kind="ExternalInput").ap()
    out_d = nc.dram_tensor("out", [1, 1], F32, kind="ExternalOutput").ap()

    mult = mybir.AluOpType.mult
    ss_eng, ii_eng = _mk_assign(pat_act_ss, pat_gp_ss, pat_gp_ii)

    with tile.TileContext(nc) as tc, ExitStack() as ctx:
        spool = ctx.enter_context(tc.tile_pool(name="spool", bufs=bufs))
        ipool = ctx.enter_context(tc.tile_pool(name="ipool", bufs=bufs))
        stats = ctx.enter_context(tc.tile_pool(name="stats", bufs=1))
        ppool = ctx.enter_context(tc.psum_pool(name="ppool", bufs=1))

        dot_all = stats.tile([P, slices], F32)
        ss_all = stats.tile([P, slices], F32)
        ii_all = stats.tile([P, slices], F32)
        dve_scr = stats.tile([P, D], F16)
        act_scr = stats.tile([P, D], F16)
        gp_scr = stats.tile([P, D], F16)
        ones = stats.tile([P, 1], F32)
        nc.vector.memset(ones[:], 1.0)

        c = 0
        r0 = 0
        for seg in seg_schedule:
            nrows = seg * P
            s_seg = s_d[r0 : r0 + nrows, :].rearrange("(p j) d -> p j d", p=P, j=seg)
            im_seg = im_d[r0 : r0 + nrows, :].rearrange("(p j) d -> p j d", p=P, j=seg)
            r0 += nrows
            st = spool.tile([P, seg, D], F16, name="st", tag="st")
            nc.sync.dma_start(st[:], s_seg)
            it = ipool.tile([P, seg, D], F16, name="it", tag="it")
            nc.sync.dma_start(it[:], im_seg)
            for j in range(seg):
                # dot: always DVE
                nc.vector.scalar_tensor_tensor(
                    out=dve_scr[:], in0=st[:, j, :], scalar=1.0, in1=it[:, j, :],
                    op0=mult, op1=mult,
                    accum_out=dot_all[:, c : c + 1],
                )
                # ss
                if ss_eng[c] == "act":
                    nc.scalar.activation(
                        out=act_scr[:], in_=st[:, j, :],
                        func=mybir.ActivationFunctionType.Square,
                        accum_out=ss_all[:, c : c + 1],
                    )
                elif ss_eng[c] == "gp":
                    nc.gpsimd.scalar_tensor_tensor(
                        out=gp_scr[:], in0=st[:, j, :], scalar=1.0, in1=st[:, j, :],
                        op0=mult, op1=mult,
                        accum_out=ss_all[:, c : c + 1],
                    )
                else:
                    nc.vector.scalar_tensor_tensor(
                        out=dve_scr[:], in0=st[:, j, :], scalar=1.0, in1=st[:, j, :],
                        op0=mult, op1=mult,
                        accum_out=ss_all[:, c : c + 1],
                    )
                # ii
                if ii_eng[c] == "gp":
                    nc.gpsimd.scalar_tensor_tensor(
                        out=gp_scr[:], in0=it[:, j, :], scalar=1.0, in1=it[:, j, :],
                        op0=mult, op1=mult,
                        accum_out=ii_all[:, c : c + 1],
                    )
                else:
                    nc.vector.scalar_tensor_tensor(
                        out=dve_scr[:], in0=it[:, j, :], scalar=1.0, in1=it[:, j, :],
                        op0=mult, op1=mult,
                        accum_out=ii_all[:, c : c + 1],
                    )
                c += 1

        # tail: loss_p[p] = -sum_c dot_c / sqrt(ss_c * ii_c)
        prod = stats.tile([P, slices], F32)
        nc.vector.tensor_tensor(out=prod[:], in0=ss_all[:], in1=ii_all[:], op=mult)
        rec = stats.tile([P, slices], F32)
        nc.vector.reciprocal(rec[:], prod[:])
        rsq = stats.tile([P, slices], F32)
        nc.scalar.sqrt(rsq[:], rec[:])
        fin_scr = stats.tile([P, slices], F32)
        loss_p = stats.tile([P, 1], F32)
        nc.vector.scalar_tensor_tensor(
            out=fin_scr[:], in0=dot_all[:], scalar=-1.0, in1=rsq[:],
            op0=mult, op1=mult,
            accum_out=loss_p[:],
        )
        # cross-partition reduce on the (idle) PE: ones^T @ loss_p -> [1,1]
        acc = ppool.tile([1, 1], F32)
        nc.tensor.matmul(acc[:], ones[:], loss_p[:])
        scal = stats.tile([1, 1], F32)
        nc.scalar.copy(scal[:], acc[:])
        nc.sync.dma_start(out_d, scal[:])

    nc.compile()
    return nc


_compiled = None


def _get_nc():
    global _compiled
    if _compiled is None:
        _compiled = _build()
    return _compiled


def _run(s, im, nc=None, **kw):
    """Cast fp16, shard, run on 8 cores, return BassKernelResults."""
    s16 = np.ascontiguousarray(np.asarray(s, dtype=np.float32).astype(np.float16))
    im16 = np.ascontiguousarray(np.asarray(im, dtype=np.float32).astype(np.float16))
    assert s16.shape == (N, D) and im16.shape == (N, D)
    if nc is None:
        nc = _get_nc()
    in_maps = [
        {"s": s16[c * ROWS : (c + 1) * ROWS], "im": im16[c * ROWS : (c + 1) * ROWS]}
        for c in range(N_CORES)
    ]
    bkr = run_bass_kernel_spmd(nc, in_maps, core_ids=list(range(N_CORES)), **kw)
    return bkr


def kernel(s, im, temp=None, **_):
    bkr = _run(s, im)
    total = np.float64(0.0)
    for r in bkr.results:
        total += np.float64(r["out"].reshape(-1)[0])
    return np.float32(total)
